# revision 11
# baseline (speedup 1.0000x reference)
"""HRR self-attention (causal holographic binding) on 8 Trainium2 cores.

Math (per batch b, head h, reference semantics):
    qkv = x @ w_qkv ; q,k,v heads of HD=128
    fq,fk,fv = fft(q|k|v, axis=-1)          (length-128 FFT == matmul with DFT matrix)
    kv   = cumsum(fk*fv, axis=seq)          (causal binding)
    vals = ifft(kv * conj(fq)).real
    out  = vals @ w_out

Implementation notes:
  * FFT/iFFT are 128x128 matmuls (HD == 128 == PE tile).  Real-input FFT is
    conjugate-symmetric; the packings below make the causal cumsum ONE
    full-height tensor_tensor_scan and the binding products TWO full-height
    element-wise muls:
      fk  = Gk^T k  : [Re 0..63 | ReNyq | Im 1..63]
      R_k : [Re 0..63 | ReNyq | Re 1..63]   (rows copied from fk via DMA)
      I_k : [ * | Im 1..63 | * | Im 1..63]  (* rows hit exact-zero partner rows)
      M1  = Gk^T v  : [Re 0..63 | ReNyq | Im 1..63]
      M2  = GM2^T v : [Im 0..63 |   0   | -Re 1..63]
      scan state = (R_k*M1 + state) - (I_k*M2)   per token
        rows 0..63 : cumsum(ReK ReV - ImK ImV)   = Re(kv)
        row  64    : cumsum(NyqK NyqV)           = Nyq(kv)
        rows 65..  : cumsum(ReK ImV + ImK ReV)   = Im(kv)
    Unbinding: fq = Gk^T q, fqs = swap(fq) (DMA row-swap; its rows 0/64 are
    annihilated by exact-zero rows of A2), inverse via A1|A2 matmuls.
  * Sharding: core c = 2*b + g handles batch b, heads 4g..4g+3.  Each core
    emits a partial out^T; the host sums the pair of partials per batch.
  * Emission is software-pipelined per head-slot s (= 4*chunk + head):
    proj(s) matmuls interleave with spectra(s-1) matmuls + DVE bind/scan,
    ifft(s-2), and the chunk output projection trails two slots.  PSUM is
    8 single-bank tiles (proj 3 + spectra 3 + ifft/out 2) so the PE never
    waits on PSUM->SBUF drains.  Weight DMAs are interleaved per (k, head)
    with the first x chunk so the PE ramps with the DMA stream.
  * All matmuls fp16 (fp32 PSUM).  DFT matrices pre-scaled by 1/16; host
    undoes the net scale.
"""

import numpy as np

B, S, D, H = 4, 4096, 1024, 8
HD = 128
NCORES = 8
HPC = H // 2            # heads per core
T = 512                 # token chunk (PSUM bank = 512 fp32)
NT = S // T
KK = D // 128           # contraction tiles for the qkv projection
NSLOT = NT * HPC        # 32 head-slots
FS = 16.0               # scale folded into each forward DFT matrix
SV = 16.0               # vals stored as vals/SV
SO = 16.0               # outT stored as out/SO  (host multiplies back)


def _build_consts():
    """Forward packed DFT matrices [Gk|GR|GI|GM2|Gs0] and inverse [A1|A2].

    Column j of each forward matrix produces packed row j (out = G^T x):
      Gk : [cos | nyq | -sin]   -> [Re | ReNyq | Im]   (M1, fq)
      GR : [cos | nyq |  cos]   -> [Re | ReNyq | Re]   (R_k)
      GI : [-sin |  0 | -sin]   -> [Im |   0   | Im]   (I_k)
      GM2: [-sin |  0 | -cos]   -> [Im |   0   | -Re]  (M2)
      Gs0: [-sin |  0 |  cos]   -> [Im |   0   | Re]   (fqs)
    """
    n = HD
    a = np.arange(n)
    cos_aj = np.cos(2 * np.pi * np.outer(a, np.arange(64)) / n)   # [a, j]
    sin_aj = np.sin(2 * np.pi * np.outer(a, np.arange(64)) / n)
    nyq = np.where(a % 2 == 0, 1.0, -1.0)              # (-1)^a

    def fwd(re_cols, col64, im_cols):
        M = np.zeros((n, n))
        M[:, :64] = re_cols
        M[:, 64] = col64
        M[:, 65:] = im_cols[:, 1:]
        return M

    Gk = fwd(cos_aj, nyq, -sin_aj)
    GR = fwd(cos_aj, nyq, cos_aj)
    GI = fwd(-sin_aj, 0.0, -sin_aj)
    GM2 = fwd(-sin_aj, 0.0, -cos_aj)
    Gs0 = fwd(-sin_aj, 0.0, cos_aj)

    # inverse: vals_n = sum_p A1[p,n] P1[p] + A2[p,n] P2[p]
    cos_jn = np.cos(2 * np.pi * np.outer(np.arange(64), a) / n)   # [j, n]
    sin_jn = np.sin(2 * np.pi * np.outer(np.arange(64), a) / n)
    w = np.full(64, 2.0)
    w[0] = 1.0
    A1 = np.zeros((n, n))
    A1[:64, :] = w[:, None] * cos_jn / n
    A1[64, :] = np.where(np.arange(n) % 2 == 0, 1.0, -1.0) / n    # Nyquist (-1)^n
    A1[65:, :] = 2.0 * cos_jn[1:] / n
    A2 = np.zeros((n, n))
    A2[:64, :] = 2.0 * sin_jn / n
    A2[64, :] = 0.0
    A2[65:, :] = -2.0 * sin_jn[1:] / n

    Amul = FS ** 3 / SV
    gmat = np.concatenate(
        [Gk / FS, GR / FS, GI / FS, GM2 / FS, Gs0 / FS], axis=1
    ).astype(np.float16)
    amat = np.concatenate([A1 * Amul, A2 * Amul], axis=1).astype(np.float16)
    return gmat, amat


def _build_program():
    import concourse.bass as bass
    import concourse.bacc as bacc
    import concourse.mybir as mybir
    import concourse.tile as tile

    f16 = mybir.dt.float16
    f32 = mybir.dt.float32
    add = mybir.AluOpType.add
    sub = mybir.AluOpType.subtract

    nc = bacc.Bacc("TRN2", target_bir_lowering=False, debug=False)
    xT = nc.dram_tensor("xT", [D, S], f16, kind="ExternalInput").ap()
    wq = nc.dram_tensor("wq", [D, 3 * HPC * 128], f16, kind="ExternalInput").ap()
    wo = nc.dram_tensor("wo", [HPC * 128, D], f16, kind="ExternalInput").ap()
    gmat = nc.dram_tensor("gmat", [128, 640], f16, kind="ExternalInput").ap()
    amat = nc.dram_tensor("amat", [128, 256], f16, kind="ExternalInput").ap()
    outT = nc.dram_tensor("outT", [D, S], f16, kind="ExternalOutput").ap()

    GK, GRC, GIC, GM2C, GS0 = 0, 128, 256, 384, 512   # gmat column offsets
    # spect tile free-dim slices (x T columns each)
    R_, I_, M1_, M2_, FQ_, FQS_ = range(6)

    with tile.TileContext(nc) as tc:
        with (
            tc.tile_pool(name="consts", bufs=1) as cpool,
            tc.tile_pool(name="xin", bufs=2) as xpool,
            tc.tile_pool(name="qkvp", bufs=2) as qkvpool,
            tc.tile_pool(name="spectp", bufs=2) as spool,
            tc.tile_pool(name="pkp", bufs=2) as pkpool,
            tc.tile_pool(name="kvp", bufs=2) as kvpool,
            tc.tile_pool(name="p12p", bufs=2) as p12pool,
            tc.tile_pool(name="valp", bufs=2) as vpool,
            tc.tile_pool(name="otp", bufs=3) as otpool,
            tc.tile_pool(name="psP", bufs=3, space="PSUM") as psP,
            tc.tile_pool(name="psS", bufs=3, space="PSUM") as psS,
            tc.tile_pool(name="psX", bufs=2, space="PSUM") as psX,
        ):
            xk_tiles = {}      # (t, k) -> tile

            def emit_xdma(t, interleave=None):
                for k in range(KK):
                    xt = xpool.tile([128, T], f16, tag=f"xk{k}", name=f"x_{t}_{k}")
                    nc.sync.dma_start(out=xt, in_=xT[k * 128:(k + 1) * 128,
                                                    t * T:(t + 1) * T])
                    xk_tiles[(t, k)] = xt
                    if interleave is not None:
                        interleave(k)

            # weights streamed per (k, head) so slot 0 can start early
            wq_sb = {}

            def emit_wq(k, h):
                wqt = cpool.tile([128, 3 * 128], f16, name=f"wq{k}_{h}")
                nc.sync.dma_start(
                    out=wqt, in_=wq[k * 128:(k + 1) * 128,
                                    h * 384:(h + 1) * 384])
                wq_sb[(k, h)] = wqt

            emit_xdma(0, interleave=lambda k: emit_wq(k, 0))
            for k in range(KK):
                emit_wq(k, 1)
            g_sb = cpool.tile([128, 640], f16, name="g_sb")
            nc.sync.dma_start(out=g_sb, in_=gmat)
            a_sb = cpool.tile([128, 256], f16, name="a_sb")
            nc.sync.dma_start(out=a_sb, in_=amat)
            for h in (2, 3):
                for k in range(KK):
                    emit_wq(k, h)
            wo_sb = []
            for h in range(HPC):
                wot = cpool.tile([128, D], f16, name=f"wo{h}")
                nc.sync.dma_start(out=wot, in_=wo[h * 128:(h + 1) * 128, :])
                wo_sb.append(wot)

            qkv_sb = {}        # s -> [128, 3T] (q|k|v)
            spect_sb = {}      # s -> [128, 7T] (R|I|M1|M2|fq|fqs|fk)
            kv_cur = {}        # h -> latest kv tile
            p12_sb = {}        # s -> [128, 2T]
            vals_sb = {}       # s -> [128, T]

            def sl(i):
                return slice(i * T, (i + 1) * T)

            def emit_proj_comp(s, comp, ps_tile):
                t, h = divmod(s, HPC)
                for k in range(KK):
                    nc.tensor.matmul(
                        ps_tile,
                        lhsT=wq_sb[(k, h)][:, comp * 128:(comp + 1) * 128],
                        rhs=xk_tiles[(t, k)],
                        start=(k == 0),
                        stop=(k == KK - 1),
                    )

            def emit_spect_mm(s, gcol, comp):
                pst = psS.tile([128, T], f32, tag="S", name=f"psS_{s}_{gcol}_{comp}")
                nc.tensor.matmul(pst, lhsT=g_sb[:, gcol:gcol + 128],
                                 rhs=qkv_sb[s][:, sl(comp)])
                return pst

            def stage2a_q(s):
                """R_k, I_k matmuls + DVE copies (needs qkv k slice of s)."""
                spect_sb[s] = spool.tile([128, 6 * T], f16, tag="spect",
                                         name=f"spect_{s}")
                sp = spect_sb[s]
                psr = emit_spect_mm(s, GRC, 1)
                psi = emit_spect_mm(s, GIC, 1)
                nc.vector.tensor_copy(sp[:, sl(R_)], psr)
                nc.vector.tensor_copy(sp[:, sl(I_)], psi)

            def stage2a_k(s):
                """M1, M2 matmuls + ACT copies."""
                sp = spect_sb[s]
                psm1 = emit_spect_mm(s, GK, 2)
                psm2 = emit_spect_mm(s, GM2C, 2)
                nc.scalar.copy(sp[:, sl(M1_)], psm1)
                nc.scalar.copy(sp[:, sl(M2_)], psm2)

            def stage2a_v(s):
                """fq, fqs matmuls + bind products, scan, unbind (DVE)."""
                t, h = divmod(s, HPC)
                sp = spect_sb[s]
                psfq = emit_spect_mm(s, GK, 0)
                psfqs = emit_spect_mm(s, GS0, 0)
                nc.vector.tensor_copy(sp[:, sl(FQ_)], psfq)
                nc.vector.tensor_copy(sp[:, sl(FQS_)], psfqs)
                pk = pkpool.tile([128, 2 * T], f16, tag="pk", name=f"pk_{s}")
                nc.vector.tensor_mul(pk[:, 0:T], sp[:, sl(R_)], sp[:, sl(M1_)])
                nc.vector.tensor_mul(pk[:, T:2 * T], sp[:, sl(I_)], sp[:, sl(M2_)])
                kvt = kvpool.tile([128, T], f16, tag=f"kv{h}", name=f"kv_{s}")
                init = 0.0 if t == 0 else kv_cur[h][:, T - 1:T]
                nc.vector.tensor_tensor_scan(
                    kvt, pk[:, 0:T], pk[:, T:2 * T], init, add, sub)
                kv_cur[h] = kvt
                p12 = p12pool.tile([128, 2 * T], f16, tag="p12", name=f"p12_{s}")
                nc.vector.tensor_mul(p12[:, 0:T], kvt, sp[:, sl(FQ_)])
                nc.vector.tensor_mul(p12[:, T:2 * T], kvt, sp[:, sl(FQS_)])
                p12_sb[s] = p12

            def stage2b(s):
                """ifft matmuls + vals copy."""
                p12 = p12_sb.pop(s)
                _, h = divmod(s, HPC)
                psval = psX.tile([128, T], f32, tag="X", name=f"psval_{s}")
                nc.tensor.matmul(psval, lhsT=a_sb[:, 0:128], rhs=p12[:, 0:T],
                                 start=True, stop=False)
                nc.tensor.matmul(psval, lhsT=a_sb[:, 128:256],
                                 rhs=p12[:, T:2 * T], start=False, stop=True)
                vt = vpool.tile([128, T], f16, tag=f"v{h}", name=f"vals_{s}")
                nc.scalar.copy(vt, psval)
                vals_sb[s] = vt

            pending_out = []   # (t, od) output-projection pieces not yet emitted

            def emit_outpiece(t, od, engine):
                ps_out = psX.tile([128, T], f32, tag="X", name=f"pso_{t}_{od}")
                for hh in range(HPC):
                    nc.tensor.matmul(
                        ps_out,
                        lhsT=wo_sb[hh][:, od * 128:(od + 1) * 128],
                        rhs=vals_sb[t * HPC + hh],
                        start=(hh == 0),
                        stop=(hh == HPC - 1),
                    )
                ott = otpool.tile([128, T], f16, tag="ot", name=f"ot_{t}_{od}")
                engine(ott, ps_out)
                nc.sync.dma_start(
                    out=outT[od * 128:(od + 1) * 128, t * T:(t + 1) * T],
                    in_=ott)

            for s in range(NSLOT):
                t, h = divmod(s, HPC)
                if h == 2 and t + 1 < NT:
                    emit_xdma(t + 1)
                prv = s - 1 if s >= 1 else None
                last = s == NSLOT - 1

                psq = psP.tile([128, T], f32, tag="P", name=f"psq_{s}")
                emit_proj_comp(s, 0, psq)
                nc.scalar.copy(qkv_sb.setdefault(
                    s, qkvpool.tile([128, 3 * T], f16, tag="qkv",
                                    name=f"qkv_{s}"))[:, 0:T], psq)
                if prv is not None:
                    stage2a_q(prv)

                psk = psP.tile([128, T], f32, tag="P", name=f"psk_{s}")
                emit_proj_comp(s, 1, psk)
                nc.scalar.copy(qkv_sb[s][:, T:2 * T], psk)
                if prv is not None:
                    stage2a_k(prv)

                psv = psP.tile([128, T], f32, tag="P", name=f"psv_{s}")
                emit_proj_comp(s, 2, psv)
                nc.scalar.copy(qkv_sb[s][:, 2 * T:3 * T], psv)
                if prv is not None:
                    stage2a_v(prv)
                # two output-projection pieces per slot (smooths ACT + psX load)
                for _ in range(2):
                    if pending_out:
                        emit_outpiece(*pending_out.pop(0), nc.scalar.copy)
                if s >= 2:
                    stage2b(s - 2)
                    bt, bh = divmod(s - 2, HPC)
                    if bh == HPC - 1:   # chunk bt's vals complete
                        pending_out.extend((bt, od) for od in range(D // 128))

                if last:   # drain the pipeline with minimal lag
                    stage2a_q(s)
                    stage2a_k(s)
                    stage2a_v(s)
                    stage2b(s - 1)
                    stage2b(s)
                    pending_out.extend((NT - 1, od) for od in range(D // 128))
                    for i, piece in enumerate(pending_out):
                        emit_outpiece(*piece,
                                      nc.scalar.copy if i % 2 else
                                      nc.vector.tensor_copy)
                    pending_out.clear()
    nc.compile()
    return nc


def _make_in_maps(x, w_qkv, w_out):
    gmat, amat = _build_consts()
    x16 = x.astype(np.float16)
    wq16 = w_qkv.astype(np.float16)
    wo16 = (w_out * (SV / SO)).astype(np.float16)
    in_maps = []
    for c in range(NCORES):
        b, g = divmod(c, 2)
        heads = range(4 * g, 4 * g + 4)
        wq_cols = np.concatenate(
            [wq16[:, comp * D + h * 128: comp * D + (h + 1) * 128]
             for h in heads for comp in range(3)], axis=1)
        wo_rows = np.concatenate(
            [wo16[h * 128:(h + 1) * 128, :] for h in heads], axis=0)
        in_maps.append({
            "xT": np.ascontiguousarray(x16[b].T),
            "wq": np.ascontiguousarray(wq_cols),
            "wo": np.ascontiguousarray(wo_rows),
            "gmat": gmat,
            "amat": amat,
        })
    return in_maps


_NC_CACHE = None


def _get_program():
    global _NC_CACHE
    if _NC_CACHE is None:
        _NC_CACHE = _build_program()
    return _NC_CACHE


def kernel(x, w_qkv, w_out, _trace=False, _results_out=None):
    import sys
    if "/opt/trn_rl_repo" not in sys.path:
        sys.path.insert(0, "/opt/trn_rl_repo")
    from concourse.bass_utils import run_bass_kernel_spmd

    x = np.asarray(x)
    w_qkv = np.asarray(w_qkv)
    w_out = np.asarray(w_out)
    nc = _get_program()
    in_maps = _make_in_maps(x, w_qkv, w_out)
    res = run_bass_kernel_spmd(nc, in_maps, list(range(NCORES)), trace=_trace)
    if _results_out is not None:
        _results_out.append(res)
    out = np.empty((B, S, D), np.float32)
    for b in range(B):
        p0 = res.results[2 * b]["outT"].astype(np.float32)
        p1 = res.results[2 * b + 1]["outT"].astype(np.float32)
        out[b] = (p0 + p1).T * SO
    return out


# revision 15
# speedup vs baseline: 1.0421x; 1.0421x over previous
"""HRR self-attention (causal holographic binding) on 8 Trainium2 cores.

Math (per batch b, head h, reference semantics):
    qkv = x @ w_qkv ; q,k,v heads of HD=128
    fq,fk,fv = fft(q|k|v, axis=-1)          (length-128 FFT == matmul with DFT matrix)
    kv   = cumsum(fk*fv, axis=seq)          (causal binding)
    vals = ifft(kv * conj(fq)).real
    out  = vals @ w_out

Implementation notes:
  * FFT/iFFT are 128x128 matmuls (HD == 128 == PE tile).  Real-input FFT is
    conjugate-symmetric; the packings below make the causal cumsum ONE
    full-height tensor_tensor_scan and the binding products TWO full-height
    element-wise muls:
      fk  = Gk^T k  : [Re 0..63 | ReNyq | Im 1..63]
      R_k : [Re 0..63 | ReNyq | Re 1..63]   (rows copied from fk via DMA)
      I_k : [ * | Im 1..63 | * | Im 1..63]  (* rows hit exact-zero partner rows)
      M1  = Gk^T v  : [Re 0..63 | ReNyq | Im 1..63]
      M2  = GM2^T v : [Im 0..63 |   0   | -Re 1..63]
      scan state = (R_k*M1 + state) - (I_k*M2)   per token
        rows 0..63 : cumsum(ReK ReV - ImK ImV)   = Re(kv)
        row  64    : cumsum(NyqK NyqV)           = Nyq(kv)
        rows 65..  : cumsum(ReK ImV + ImK ReV)   = Im(kv)
    Unbinding: fq = Gk^T q, fqs = swap(fq) (DMA row-swap; its rows 0/64 are
    annihilated by exact-zero rows of A2), inverse via A1|A2 matmuls.
  * Sharding: core c = 2*b + g handles batch b, heads 4g..4g+3.  Each core
    emits a partial out^T; the host sums the pair of partials per batch.
  * Emission is software-pipelined per head-slot s (= 4*chunk + head):
    proj(s) matmuls interleave with spectra(s-1) matmuls + DVE bind/scan,
    ifft(s-2), and the chunk output projection trails two slots.  PSUM is
    8 single-bank tiles (proj 3 + spectra 3 + ifft/out 2) so the PE never
    waits on PSUM->SBUF drains.  Weight DMAs are interleaved per (k, head)
    with the first x chunk so the PE ramps with the DMA stream.
  * All matmuls fp16 (fp32 PSUM).  DFT matrices pre-scaled by 1/16; host
    undoes the net scale.
"""

import numpy as np

B, S, D, H = 4, 4096, 1024, 8
HD = 128
NCORES = 8
HPC = H // 2            # heads per core
T = 512                 # token chunk (PSUM bank = 512 fp32)
NT = S // T
KK = D // 128           # contraction tiles for the qkv projection
NSLOT = NT * HPC        # 32 head-slots
FS = 16.0               # scale folded into each forward DFT matrix
SV = 16.0               # vals stored as vals/SV
SO = 16.0               # outT stored as out/SO  (host multiplies back)


def _build_consts():
    """Forward packed DFT matrices [Gm|Gfk|Gm2] and inverse [A1|A2].

    Interleaved packing: bin j (0..63) lives in quadrant j//16 at
    R-row(j) = 32*(j//16) + j%16 (real part) and I-row(j) = R-row(j)+16
    (imag part).  I-row(0) is the DC/Nyquist helper row.  With this layout
    the Re<->Im derived spectra (R_k, I_k, fqs) are quadrant-local
    partition permutes = single DVE stream_shuffle ops.

      Gm  (M1, fq): R-rows = Re bins, I-row(0) = ReNyq, I-rows = Im bins
      Gfk (fk)    : same but I-row(0) col = (1 - (-1)^a)  [= Re0 - Nyq]
      Gm2 (M2)    : R-rows = Im bins, I-row(0) = ReNyq, I-rows = -Re bins

      R_k = shuffle(fk, [0..15,0..15])    I_k = shuffle(fk, [16..31,16..31])
      fqs = shuffle(fq, [16..31,0..15])
      scan state = (R_k*M1 + state) - (I_k*M2):
        R-rows   : cumsum(ReK ReV - ImK ImV) = Re(kv)   (R-row(0): DC, exact
                   because M2's R-row(0) = Im0(v) = 0)
        I-row(0) : cumsum(Re0K NyqV - (Re0K-NyqK) NyqV) = Nyq(kv)
        I-rows   : cumsum(ReK ImV + ImK ReV) = Im(kv)
      Unbind p2 = kv*fqs rows R-row(0)/I-row(0) are annihilated by exact-zero
      rows of A2.
    """
    n = HD
    a = np.arange(n)
    nyq = np.where(a % 2 == 0, 1.0, -1.0)              # (-1)^a

    def rrow(j):
        return 32 * (j // 16) + j % 16

    def irow(j):
        return rrow(j) + 16

    def fwd(re_fn, i0_col, im_fn):
        M = np.zeros((n, n))
        for j in range(64):
            M[:, rrow(j)] = re_fn(j)
        M[:, irow(0)] = i0_col
        for j in range(1, 64):
            M[:, irow(j)] = im_fn(j)
        return M

    def cos(j):
        return np.cos(2 * np.pi * a * j / n)

    def sin(j):
        return np.sin(2 * np.pi * a * j / n)

    Gm = fwd(cos, nyq, lambda j: -sin(j))
    Gfk = fwd(cos, cos(0) - nyq, lambda j: -sin(j))
    Gm2 = fwd(lambda j: -sin(j), nyq, lambda j: -cos(j))

    # inverse: vals_n = sum_p A1[p,n] P1[p] + A2[p,n] P2[p]
    A1 = np.zeros((n, n))
    A2 = np.zeros((n, n))
    for j in range(64):
        w = 1.0 if j == 0 else 2.0
        A1[rrow(j)] = w * np.cos(2 * np.pi * j * a / n) / n
        A2[rrow(j)] = 2.0 * np.sin(2 * np.pi * j * a / n) / n   # j=0 -> 0
    A1[irow(0)] = nyq / n
    A2[irow(0)] = 0.0
    for j in range(1, 64):
        A1[irow(j)] = 2.0 * np.cos(2 * np.pi * j * a / n) / n
        A2[irow(j)] = -2.0 * np.sin(2 * np.pi * j * a / n) / n

    Amul = FS ** 3 / SV
    gmat = np.concatenate(
        [Gm / FS, Gfk / FS, Gm2 / FS], axis=1).astype(np.float16)
    amat = np.concatenate([A1 * Amul, A2 * Amul], axis=1).astype(np.float16)
    return gmat, amat


def _build_program():
    import concourse.bass as bass
    import concourse.bacc as bacc
    import concourse.mybir as mybir
    import concourse.tile as tile

    f16 = mybir.dt.float16
    f32 = mybir.dt.float32
    add = mybir.AluOpType.add
    sub = mybir.AluOpType.subtract

    nc = bacc.Bacc("TRN2", target_bir_lowering=False, debug=False)
    xT = nc.dram_tensor("xT", [D, S], f16, kind="ExternalInput").ap()
    wq = nc.dram_tensor("wq", [D, 3 * HPC * 128], f16, kind="ExternalInput").ap()
    wo = nc.dram_tensor("wo", [HPC * 128, D], f16, kind="ExternalInput").ap()
    gmat = nc.dram_tensor("gmat", [128, 384], f16, kind="ExternalInput").ap()
    amat = nc.dram_tensor("amat", [128, 256], f16, kind="ExternalInput").ap()
    outT = nc.dram_tensor("outT", [D, S], f16, kind="ExternalOutput").ap()

    GM, GFK, GM2C = 0, 128, 256   # gmat column offsets
    # spect tile free-dim slices (x T columns each)
    R_, I_, M1_, M2_, FQ_, FQS_, FK_ = range(7)
    DUP_R = list(range(16)) * 2          # stream_shuffle masks (per quadrant)
    DUP_I = list(range(16, 32)) * 2
    SWAP_RI = list(range(16, 32)) + list(range(16))

    with tile.TileContext(nc) as tc:
        with (
            tc.tile_pool(name="consts", bufs=1) as cpool,
            tc.tile_pool(name="xin", bufs=2) as xpool,
            tc.tile_pool(name="qkvp", bufs=2) as qkvpool,
            tc.tile_pool(name="spectp", bufs=2) as spool,
            tc.tile_pool(name="pkp", bufs=2) as pkpool,
            tc.tile_pool(name="kvp", bufs=2) as kvpool,
            tc.tile_pool(name="p12p", bufs=2) as p12pool,
            tc.tile_pool(name="valp", bufs=2) as vpool,
            tc.tile_pool(name="otp", bufs=3) as otpool,
            tc.tile_pool(name="psP", bufs=3, space="PSUM") as psP,
            tc.tile_pool(name="psS", bufs=3, space="PSUM") as psS,
            tc.tile_pool(name="psX", bufs=2, space="PSUM") as psX,
        ):
            xk_tiles = {}      # (t, k) -> tile

            def emit_xdma(t, interleave=None):
                for k in range(KK):
                    xt = xpool.tile([128, T], f16, tag=f"xk{k}", name=f"x_{t}_{k}")
                    nc.sync.dma_start(out=xt, in_=xT[k * 128:(k + 1) * 128,
                                                    t * T:(t + 1) * T])
                    xk_tiles[(t, k)] = xt
                    if interleave is not None:
                        interleave(k)

            # weights streamed per (k, head) so slot 0 can start early
            wq_sb = {}

            def emit_wq(k, h):
                wqt = cpool.tile([128, 3 * 128], f16, name=f"wq{k}_{h}")
                nc.sync.dma_start(
                    out=wqt, in_=wq[k * 128:(k + 1) * 128,
                                    h * 384:(h + 1) * 384])
                wq_sb[(k, h)] = wqt

            emit_xdma(0, interleave=lambda k: emit_wq(k, 0))
            for k in range(KK):
                emit_wq(k, 1)
            g_sb = cpool.tile([128, 384], f16, name="g_sb")
            nc.sync.dma_start(out=g_sb, in_=gmat)
            a_sb = cpool.tile([128, 256], f16, name="a_sb")
            nc.sync.dma_start(out=a_sb, in_=amat)
            for h in (2, 3):
                for k in range(KK):
                    emit_wq(k, h)
            wo_sb = []
            for h in range(HPC):
                wot = cpool.tile([128, D], f16, name=f"wo{h}")
                nc.sync.dma_start(out=wot, in_=wo[h * 128:(h + 1) * 128, :])
                wo_sb.append(wot)

            qkv_sb = {}        # s -> [128, 3T] (q|k|v)
            spect_sb = {}      # s -> [128, 7T] (R|I|M1|M2|fq|fqs|fk)
            kv_cur = {}        # h -> latest kv tile
            p12_sb = {}        # s -> [128, 2T]
            vals_sb = {}       # s -> [128, T]

            def sl(i):
                return slice(i * T, (i + 1) * T)

            def emit_proj_comp(s, comp, ps_tile):
                t, h = divmod(s, HPC)
                for k in range(KK):
                    nc.tensor.matmul(
                        ps_tile,
                        lhsT=wq_sb[(k, h)][:, comp * 128:(comp + 1) * 128],
                        rhs=xk_tiles[(t, k)],
                        start=(k == 0),
                        stop=(k == KK - 1),
                    )

            def emit_spect_mm(s, gcol, comp):
                pst = psS.tile([128, T], f32, tag="S", name=f"psS_{s}_{gcol}_{comp}")
                nc.tensor.matmul(pst, lhsT=g_sb[:, gcol:gcol + 128],
                                 rhs=qkv_sb[s][:, sl(comp)])
                return pst

            def stage2a_q(s):
                """fk, fq matmuls + DVE copies (needs qkv q,k slices of s)."""
                spect_sb[s] = spool.tile([128, 7 * T], f16, tag="spect",
                                         name=f"spect_{s}")
                sp = spect_sb[s]
                psfk = emit_spect_mm(s, GFK, 1)
                psfq = emit_spect_mm(s, GM, 0)
                nc.vector.tensor_copy(sp[:, sl(FK_)], psfk)
                nc.vector.tensor_copy(sp[:, sl(FQ_)], psfq)

            def stage2a_k(s):
                """M1, M2 matmuls + ACT copies; derived spectra via shuffles."""
                sp = spect_sb[s]
                psm1 = emit_spect_mm(s, GM, 2)
                psm2 = emit_spect_mm(s, GM2C, 2)
                nc.scalar.copy(sp[:, sl(M1_)], psm1)
                nc.scalar.copy(sp[:, sl(M2_)], psm2)
                nc.vector.stream_shuffle(sp[:, sl(R_)], sp[:, sl(FK_)], DUP_R)
                nc.vector.stream_shuffle(sp[:, sl(I_)], sp[:, sl(FK_)], DUP_I)
                nc.vector.stream_shuffle(sp[:, sl(FQS_)], sp[:, sl(FQ_)],
                                         SWAP_RI)

            def stage2a_v(s):
                """bind products, scan, unbind muls (DVE)."""
                t, h = divmod(s, HPC)
                sp = spect_sb[s]
                pk = pkpool.tile([128, 2 * T], f16, tag="pk", name=f"pk_{s}")
                nc.vector.tensor_mul(pk[:, 0:T], sp[:, sl(R_)], sp[:, sl(M1_)])
                nc.vector.tensor_mul(pk[:, T:2 * T], sp[:, sl(I_)], sp[:, sl(M2_)])
                kvt = kvpool.tile([128, T], f16, tag=f"kv{h}", name=f"kv_{s}")
                init = 0.0 if t == 0 else kv_cur[h][:, T - 1:T]
                nc.vector.tensor_tensor_scan(
                    kvt, pk[:, 0:T], pk[:, T:2 * T], init, add, sub)
                kv_cur[h] = kvt
                p12 = p12pool.tile([128, 2 * T], f16, tag="p12", name=f"p12_{s}")
                nc.vector.tensor_mul(p12[:, 0:T], kvt, sp[:, sl(FQ_)])
                nc.vector.tensor_mul(p12[:, T:2 * T], kvt, sp[:, sl(FQS_)])
                p12_sb[s] = p12

            def stage2b(s):
                """ifft matmuls + vals copy."""
                p12 = p12_sb.pop(s)
                _, h = divmod(s, HPC)
                psval = psX.tile([128, T], f32, tag="X", name=f"psval_{s}")
                nc.tensor.matmul(psval, lhsT=a_sb[:, 0:128], rhs=p12[:, 0:T],
                                 start=True, stop=False)
                nc.tensor.matmul(psval, lhsT=a_sb[:, 128:256],
                                 rhs=p12[:, T:2 * T], start=False, stop=True)
                vt = vpool.tile([128, T], f16, tag=f"v{h}", name=f"vals_{s}")
                nc.scalar.copy(vt, psval)
                vals_sb[s] = vt

            pending_out = []   # (t, od) output-projection pieces not yet emitted

            def emit_outpiece(t, od, engine):
                ps_out = psX.tile([128, T], f32, tag="X", name=f"pso_{t}_{od}")
                for hh in range(HPC):
                    nc.tensor.matmul(
                        ps_out,
                        lhsT=wo_sb[hh][:, od * 128:(od + 1) * 128],
                        rhs=vals_sb[t * HPC + hh],
                        start=(hh == 0),
                        stop=(hh == HPC - 1),
                    )
                ott = otpool.tile([128, T], f16, tag="ot", name=f"ot_{t}_{od}")
                engine(ott, ps_out)
                nc.sync.dma_start(
                    out=outT[od * 128:(od + 1) * 128, t * T:(t + 1) * T],
                    in_=ott)

            for s in range(NSLOT):
                t, h = divmod(s, HPC)
                if h == 2 and t + 1 < NT:
                    emit_xdma(t + 1)
                prv = s - 1 if s >= 1 else None
                last = s == NSLOT - 1

                psq = psP.tile([128, T], f32, tag="P", name=f"psq_{s}")
                emit_proj_comp(s, 0, psq)
                nc.scalar.copy(qkv_sb.setdefault(
                    s, qkvpool.tile([128, 3 * T], f16, tag="qkv",
                                    name=f"qkv_{s}"))[:, 0:T], psq)
                if prv is not None:
                    stage2a_q(prv)

                psk = psP.tile([128, T], f32, tag="P", name=f"psk_{s}")
                emit_proj_comp(s, 1, psk)
                nc.scalar.copy(qkv_sb[s][:, T:2 * T], psk)
                if prv is not None:
                    stage2a_k(prv)

                psv = psP.tile([128, T], f32, tag="P", name=f"psv_{s}")
                emit_proj_comp(s, 2, psv)
                nc.scalar.copy(qkv_sb[s][:, 2 * T:3 * T], psv)
                if prv is not None:
                    stage2a_v(prv)
                # two output-projection pieces per slot (smooths ACT + psX load)
                for _ in range(2):
                    if pending_out:
                        emit_outpiece(*pending_out.pop(0), nc.scalar.copy)
                if s >= 2:
                    stage2b(s - 2)
                    bt, bh = divmod(s - 2, HPC)
                    if bh == HPC - 1:   # chunk bt's vals complete
                        pending_out.extend((bt, od) for od in range(D // 128))

                if last:   # drain the pipeline with minimal lag
                    stage2a_q(s)
                    stage2a_k(s)
                    stage2a_v(s)
                    stage2b(s - 1)
                    stage2b(s)
                    pending_out.extend((NT - 1, od) for od in range(D // 128))
                    for i, piece in enumerate(pending_out):
                        emit_outpiece(*piece,
                                      nc.scalar.copy if i % 2 else
                                      nc.vector.tensor_copy)
                    pending_out.clear()
    nc.compile()
    return nc


def _make_in_maps(x, w_qkv, w_out):
    gmat, amat = _build_consts()
    x16 = x.astype(np.float16)
    wq16 = w_qkv.astype(np.float16)
    wo16 = (w_out * (SV / SO)).astype(np.float16)
    in_maps = []
    for c in range(NCORES):
        b, g = divmod(c, 2)
        heads = range(4 * g, 4 * g + 4)
        wq_cols = np.concatenate(
            [wq16[:, comp * D + h * 128: comp * D + (h + 1) * 128]
             for h in heads for comp in range(3)], axis=1)
        wo_rows = np.concatenate(
            [wo16[h * 128:(h + 1) * 128, :] for h in heads], axis=0)
        in_maps.append({
            "xT": np.ascontiguousarray(x16[b].T),
            "wq": np.ascontiguousarray(wq_cols),
            "wo": np.ascontiguousarray(wo_rows),
            "gmat": gmat,
            "amat": amat,
        })
    return in_maps


_NC_CACHE = None


def _get_program():
    global _NC_CACHE
    if _NC_CACHE is None:
        _NC_CACHE = _build_program()
    return _NC_CACHE


def kernel(x, w_qkv, w_out, _trace=False, _results_out=None):
    import sys
    if "/opt/trn_rl_repo" not in sys.path:
        sys.path.insert(0, "/opt/trn_rl_repo")
    from concourse.bass_utils import run_bass_kernel_spmd

    x = np.asarray(x)
    w_qkv = np.asarray(w_qkv)
    w_out = np.asarray(w_out)
    nc = _get_program()
    in_maps = _make_in_maps(x, w_qkv, w_out)
    res = run_bass_kernel_spmd(nc, in_maps, list(range(NCORES)), trace=_trace)
    if _results_out is not None:
        _results_out.append(res)
    out = np.empty((B, S, D), np.float32)
    for b in range(B):
        p0 = res.results[2 * b]["outT"].astype(np.float32)
        p1 = res.results[2 * b + 1]["outT"].astype(np.float32)
        out[b] = (p0 + p1).T * SO
    return out
